# revision 24
# baseline (speedup 1.0000x reference)
"""Trainium2 Bass kernel for nn_CrossAttention (B=4, Q=512, KV=2048, H=16 heads,
HID=1024, dh=64), sharded over 8 NeuronCores: data-parallel over batch (4) x
tensor-parallel over heads (2 groups of 8 heads).

Core c = 2*b + g handles batch b, head-group g (hidden slice g*512..g*512+512).

v2 design: all operands are pre-transposed/cast on the HOST so no on-chip
transposes are needed; matmuls run in bf16 (fp32 PSUM). Attention is
interleaved with the k/v projections per 512-row kv chunk so the ScalarE exp
work overlaps TensorE matmuls.

Per-core program:
  - qTp = Wq_g^T.T @ q^T          [hd on partitions, q free]
  - per kv chunk j (512 rows):
      kT chunk  = Wk_g^T.T @ kv^T chunk
      v chunk   = kv^T.T @ Wv_g^T  (+ ones column per head -> vA)
      per head: scoresT = kT_h.T @ qTp_h ; probsT = exp(scale*s + maskbias)
                av(+sums row) += vA_h.T @ probsT  (PSUM per chunk, fp32
                accumulated into SBUF attn_acc across chunks)
  - normalize per head-pair: recip of sums row, broadcast via tiny matmul,
    multiply -> attnT (bf16)
  - out = attnT.T @ Wo_g^T + bias ; DMA to cc_in
  - pairwise ReduceScatter(add) over q rows -> each core returns 256 q rows

kernel(**inputs) takes full inputs, shards/transposes/casts on host, runs SPMD
on cores 0-7, and reassembles the (4, 512, 1024) output.
"""

import numpy as np
import ml_dtypes

import concourse.bass as bass
import concourse.mybir as mybir
import concourse.tile as tile
from concourse import bacc
from concourse.bass_utils import run_bass_kernel_spmd

N_CORES = 8
P = 128
B, Q, KV, HID = 4, 512, 2048, 1024
HDS = 512          # head-dim slice per core (8 heads x 64)
NHEADS = 8         # heads per core
DH = 64
SCALE = 0.125      # 1/sqrt(64)
MASK_BIG = 1e30

F32 = mybir.dt.float32
BF16 = mybir.dt.bfloat16
F32R = mybir.dt.float32r

NKB = HID // P      # 8 contraction blocks
NMB = HDS // P      # 4 head-dim blocks (head pairs)
NCH = KV // 512     # 4 kv chunks


def _build(loop_k: int = 0, use_f32r: bool = True, analysis: bool = False,
           skip: tuple = ()):
    """Build the SPMD program. loop_k>0 wraps the compute in a For_i hardware
    loop (for timing); the collective + final DMAs stay outside the loop.
    analysis=True builds a 1-core, collective-free variant for TimelineSim.
    use_f32r=True -> bf16 operands (fast path); False -> f32r (debug).
    skip: timing-dissection only (results wrong): subsets of
    {"scores", "av", "norm", "out"}."""
    DT = BF16 if use_f32r else F32R
    do_proj = "proj" not in skip
    do_scores = do_proj and "scores" not in skip
    do_av = do_scores and "av" not in skip
    do_norm = do_av and "norm" not in skip
    do_out = do_proj and "out" not in skip

    nc = bacc.Bacc("TRN2", target_bir_lowering=False, debug=False,
                   num_devices=1 if analysis else N_CORES)

    qT_in = nc.dram_tensor("qT_in", [HID, Q], DT, kind="ExternalInput")
    kvT_in = nc.dram_tensor("kvT_in", [HID, KV], DT, kind="ExternalInput")
    wq_in = nc.dram_tensor("wqT", [HID, HDS], DT, kind="ExternalInput")
    wk_in = nc.dram_tensor("wkT", [HID, HDS], DT, kind="ExternalInput")
    wv_in = nc.dram_tensor("wvT", [HID, HDS], DT, kind="ExternalInput")
    wo_in = nc.dram_tensor("woT", [HDS, HID], DT, kind="ExternalInput")
    bias16_in = nc.dram_tensor("bias16", [P, KV // P], F32, kind="ExternalInput")
    biasbc_in = nc.dram_tensor("bias_bc", [P, HID], F32, kind="ExternalInput")
    onesc_in = nc.dram_tensor("onesc", [P, DH], DT, kind="ExternalInput")
    ones8_in = nc.dram_tensor("ones8", [P, NHEADS], DT, kind="ExternalInput")
    out_ext = nc.dram_tensor("out", [Q // 2, HID], F32, kind="ExternalOutput")

    cc_in = nc.dram_tensor("cc_in", [Q, HID], F32)
    cc_out = nc.dram_tensor("cc_out", [Q // 2, HID], F32)

    with tile.TileContext(nc) as tc:
        with (
            tc.tile_pool(name="persist", bufs=1) as pp,
            tc.tile_pool(name="probs", bufs=12) as prp,
            tc.tile_pool(name="small", bufs=2) as smallp,
            tc.tile_pool(name="outc", bufs=3) as ocp,
            tc.tile_pool(name="psum_proj", bufs=2, space="PSUM") as psp,
            tc.tile_pool(name="psum_s", bufs=4, space="PSUM") as pss,
            tc.tile_pool(name="psum_av", bufs=2, space="PSUM") as psav,
        ):
            def body():
                # ---- input DMAs (emission order ~= priority) ----
                wq = [pp.tile([P, HDS], DT, tag=f"wq{kb}", name=f"wq{kb}") for kb in range(NKB)]
                qT = [pp.tile([P, Q], DT, tag=f"qT{kb}", name=f"qT{kb}") for kb in range(NKB)]
                for kb in range(NKB):
                    nc.sync.dma_start(wq[kb][:], wq_in[kb * P:(kb + 1) * P, :])
                    nc.sync.dma_start(qT[kb][:], qT_in[kb * P:(kb + 1) * P, :])

                wk = [pp.tile([P, HDS], DT, tag=f"wk{kb}", name=f"wk{kb}") for kb in range(NKB)]
                wv = [pp.tile([P, HDS], DT, tag=f"wv{kb}", name=f"wv{kb}") for kb in range(NKB)]
                kvT = [pp.tile([P, KV], DT, tag=f"kvT{kb}", name=f"kvT{kb}") for kb in range(NKB)]
                for kb in range(NKB):
                    nc.sync.dma_start(wk[kb][:], wk_in[kb * P:(kb + 1) * P, :])
                for kb in range(NKB):
                    nc.sync.dma_start(kvT[kb][:, 0:512],
                                      kvT_in[kb * P:(kb + 1) * P, 0:512])
                for kb in range(NKB):
                    nc.sync.dma_start(wv[kb][:], wv_in[kb * P:(kb + 1) * P, :])

                bias16 = pp.tile([P, KV // P], F32, tag="bias16")
                nc.sync.dma_start(bias16[:], bias16_in[:, :])
                ones8 = pp.tile([P, NHEADS], DT, tag="ones8")
                nc.sync.dma_start(ones8[:], ones8_in[:, :])
                onesc = pp.tile([P, DH], DT, tag="onesc")
                nc.sync.dma_start(onesc[:], onesc_in[:, :])

                for j in range(1, NCH):
                    for kb in range(NKB):
                        nc.sync.dma_start(
                            kvT[kb][:, j * 512:(j + 1) * 512],
                            kvT_in[kb * P:(kb + 1) * P, j * 512:(j + 1) * 512])

                wo = [pp.tile([P, HID], DT, tag=f"wo{mb}", name=f"wo{mb}") for mb in range(NMB)]
                for mb in range(NMB):
                    nc.sync.dma_start(wo[mb][:], wo_in[mb * P:(mb + 1) * P, :])
                bias_bc = pp.tile([P, HID], F32, tag="bias_bc")
                nc.sync.dma_start(bias_bc[:], biasbc_in[:, :])

                # ---- q projection ----
                qTp = [pp.tile([P, Q], DT, tag=f"qTp{mb}", name=f"qTp{mb}") for mb in range(NMB)]
                for mb in range(NMB if do_proj else 0):
                    ps = psp.tile([P, 512], F32, tag="proj")
                    for kb in range(NKB):
                        nc.tensor.matmul(
                            ps[:], wq[kb][:, mb * P:(mb + 1) * P], qT[kb][:],
                            start=(kb == 0), stop=(kb == NKB - 1))
                    nc.vector.tensor_copy(out=qTp[mb][:], in_=ps[:])

                # ---- persistent attention state ----
                kT = [pp.tile([P, KV], DT, tag=f"kT{mb}", name=f"kT{mb}") for mb in range(NMB)]
                vA = [pp.tile([P, NHEADS * (DH + 1)], DT, tag=f"vA{kvb}", name=f"vA{kvb}")
                      for kvb in range(KV // P)]
                attn_acc = [pp.tile([DH + 1, Q], F32, tag=f"acc{h}", name=f"acc{h}")
                            for h in range(NHEADS)]
                attnT = [pp.tile([P, Q], DT, tag=f"attnT{mb}", name=f"attnT{mb}")
                         for mb in range(NMB)]

                # ---- kv chunks: k/v proj + attention ----
                for j in range(NCH if do_proj else 0):
                    # k-proj for this chunk
                    for mb in range(NMB):
                        ps = psp.tile([P, 512], F32, tag="proj")
                        for kb in range(NKB):
                            nc.tensor.matmul(
                                ps[:], wk[kb][:, mb * P:(mb + 1) * P],
                                kvT[kb][:, j * 512:(j + 1) * 512],
                                start=(kb == 0), stop=(kb == NKB - 1))
                        nc.vector.tensor_copy(
                            out=kT[mb][:, j * 512:(j + 1) * 512], in_=ps[:])
                    # v-proj for this chunk
                    for v4 in range(4):
                        kvb = 4 * j + v4
                        ps = psp.tile([P, 512], F32, tag="proj")
                        for kb in range(NKB):
                            nc.tensor.matmul(
                                ps[:], kvT[kb][:, kvb * P:(kvb + 1) * P],
                                wv[kb][:],
                                start=(kb == 0), stop=(kb == NKB - 1))
                        dst = vA[kvb][:].rearrange("p (h d) -> p h d", d=DH + 1)
                        nc.vector.tensor_copy(
                            out=dst[:, :, 0:DH],
                            in_=ps[:].rearrange("p (h d) -> p h d", d=DH))
                        nc.vector.tensor_copy(
                            out=dst[:, :, DH:DH + 1],
                            in_=ones8[:].rearrange("p (h o) -> p h o", o=1))

                    # attention for this chunk
                    for hp in range(NMB if do_scores else 0):
                        probs = {}
                        # scores interleaved by row-group pair (off 0 / 64)
                        # so the two 64-row matmuls can overlap on the PE
                        for k4 in range(4):
                            kvb = 4 * j + k4
                            for h2 in range(2):
                                off = h2 * DH
                                ss = pss.tile([P, Q], F32, tag="ss")
                                nc.tensor.matmul(
                                    ss[:],
                                    kT[hp][off:off + DH,
                                           kvb * P:(kvb + 1) * P],
                                    qTp[hp][off:off + DH, :],
                                    start=True, stop=True)
                                pr = prp.tile([P, Q], DT, tag="probs")
                                nc.scalar.activation(
                                    pr[:], ss[:],
                                    mybir.ActivationFunctionType.Exp,
                                    bias=bias16[:, kvb:kvb + 1], scale=SCALE)
                                probs[(h2, k4)] = pr
                        for h2 in range(2 if do_av else 0):
                            h = 2 * hp + h2
                            avp = psav.tile([DH + 1, Q], F32, tag="av")
                            for k4 in range(4):
                                kvb = 4 * j + k4
                                nc.tensor.matmul(
                                    avp[:],
                                    vA[kvb][:, h * (DH + 1):(h + 1) * (DH + 1)],
                                    probs[(h2, k4)][:],
                                    start=(k4 == 0), stop=(k4 == 3))
                            if j == 0:
                                nc.vector.tensor_copy(out=attn_acc[h][:],
                                                      in_=avp[:])
                            else:
                                nc.vector.tensor_tensor(
                                    attn_acc[h][:], avp[:], attn_acc[h][:],
                                    mybir.AluOpType.add)

                        # normalization per pair after last chunk
                        if j == NCH - 1 and do_norm:
                            recb = smallp.tile([DH + 1, Q], DT, tag="recb")
                            for h2 in range(2):
                                rf = smallp.tile([1, Q], F32,
                                                 tag=f"rf{h2}")
                                nc.vector.reciprocal(
                                    rf[:],
                                    attn_acc[2 * hp + h2][DH:DH + 1, :])
                                nc.vector.tensor_copy(
                                    out=recb[h2 * DH:h2 * DH + 1, :],
                                    in_=rf[:])
                            rps = psp.tile([P, 512], F32, tag="proj")
                            nc.tensor.matmul(rps[0:DH, :], onesc[0:1, :],
                                             recb[0:1, :],
                                             start=True, stop=True)
                            nc.tensor.matmul(rps[DH:P, :],
                                             onesc[DH:DH + 1, :],
                                             recb[DH:DH + 1, :],
                                             start=True, stop=True)
                            nc.vector.tensor_tensor(
                                attnT[hp][0:DH, :],
                                attn_acc[2 * hp][0:DH, :], rps[0:DH, :],
                                mybir.AluOpType.mult)
                            nc.vector.tensor_tensor(
                                attnT[hp][DH:P, :],
                                attn_acc[2 * hp + 1][0:DH, :], rps[DH:P, :],
                                mybir.AluOpType.mult)

                # ---- out projection + bias ----
                for qb in range(Q // P if do_out else 0):
                    for oh in range(2):
                        ps = psp.tile([P, 512], F32, tag="proj")
                        for mb in range(NMB):
                            nc.tensor.matmul(
                                ps[:], attnT[mb][:, qb * P:(qb + 1) * P],
                                wo[mb][:, oh * 512:(oh + 1) * 512],
                                start=(mb == 0), stop=(mb == NMB - 1))
                        oc = ocp.tile([P, 512], F32, tag="oc")
                        nc.vector.tensor_tensor(
                            oc[:], ps[:], bias_bc[:, oh * 512:(oh + 1) * 512],
                            mybir.AluOpType.add)
                        nc.sync.dma_start(
                            cc_in[qb * P:(qb + 1) * P,
                                  oh * 512:(oh + 1) * 512],
                            oc[:])

            if loop_k > 0:
                with tc.For_i(0, loop_k,
                              hint_engines=(mybir.EngineType.PE,
                                            mybir.EngineType.Activation,
                                            mybir.EngineType.DVE)):
                    body()
            else:
                body()

            # ---- pairwise reduce-scatter over q rows ----
            if analysis:
                nc.sync.dma_start(out_ext[:, :], cc_in[: Q // 2, :])
            else:
                nc.gpsimd.collective_compute(
                    "ReduceScatter",
                    mybir.AluOpType.add,
                    replica_groups=[[0, 1], [2, 3], [4, 5], [6, 7]],
                    ins=[cc_in.ap().opt()],
                    outs=[cc_out.ap().opt()],
                )
                nc.sync.dma_start(out_ext[:, :], cc_out[:, :])

    nc.compile()
    return nc


_CACHE = {}


def _get_nc(loop_k: int = 0, use_f32r: bool = True):
    key = (loop_k, use_f32r)
    if key not in _CACHE:
        _CACHE[key] = _build(loop_k, use_f32r)
    return _CACHE[key]


def make_in_maps(query, key_value, mask, Wq, Wk, Wv, Wo, bo, use_f32r=True):
    np_dt = ml_dtypes.bfloat16 if use_f32r else np.float32
    query = np.asarray(query, dtype=np.float32)
    key_value = np.asarray(key_value, dtype=np.float32)
    mask_f = np.asarray(mask).astype(np.float32)
    Wq = np.asarray(Wq, dtype=np.float32)
    Wk = np.asarray(Wk, dtype=np.float32)
    Wv = np.asarray(Wv, dtype=np.float32)
    Wo = np.asarray(Wo, dtype=np.float32)
    bo = np.asarray(bo, dtype=np.float32)

    onesc = np.ones((P, DH), dtype=np_dt)
    ones8 = np.ones((P, NHEADS), dtype=np_dt)

    in_maps = []
    for c in range(N_CORES):
        b, g = c // 2, c % 2
        sl = slice(g * HDS, (g + 1) * HDS)
        bias16 = ((mask_f[b] - 1.0) * MASK_BIG).reshape(KV // P, P).T
        bias_bc = np.broadcast_to(
            bo if g == 0 else np.zeros_like(bo), (P, HID))
        in_maps.append({
            "qT_in": np.ascontiguousarray(query[b].T).astype(np_dt),
            "kvT_in": np.ascontiguousarray(key_value[b].T).astype(np_dt),
            "wqT": np.ascontiguousarray(Wq[sl, :].T).astype(np_dt),
            "wkT": np.ascontiguousarray(Wk[sl, :].T).astype(np_dt),
            "wvT": np.ascontiguousarray(Wv[sl, :].T).astype(np_dt),
            "woT": np.ascontiguousarray(Wo[:, sl].T).astype(np_dt),
            "bias16": np.ascontiguousarray(bias16),
            "bias_bc": np.ascontiguousarray(bias_bc),
            "onesc": onesc,
            "ones8": ones8,
        })
    return in_maps


def kernel(query, key_value, mask, Wq, Wk, Wv, Wo, bo):
    nc = _get_nc(0, True)
    in_maps = make_in_maps(query, key_value, mask, Wq, Wk, Wv, Wo, bo)
    res = run_bass_kernel_spmd(nc, in_maps, list(range(N_CORES))).results
    out = np.empty((B, Q, HID), dtype=np.float32)
    for b_i in range(B):
        out[b_i, : Q // 2] = res[2 * b_i]["out"]
        out[b_i, Q // 2:] = res[2 * b_i + 1]["out"]
    return out


# revision 26
# speedup vs baseline: 1.0861x; 1.0861x over previous
"""Trainium2 Bass kernel for nn_CrossAttention (B=4, Q=512, KV=2048, H=16 heads,
HID=1024, dh=64), sharded over 8 NeuronCores: data-parallel over batch (4) x
tensor-parallel over heads (2 groups of 8 heads).

Core c = 2*b + g handles batch b, head-group g (hidden slice g*512..g*512+512).

v2 design: all operands are pre-transposed/cast on the HOST so no on-chip
transposes are needed; matmuls run in bf16 (fp32 PSUM). Attention is
interleaved with the k/v projections per 512-row kv chunk so the ScalarE exp
work overlaps TensorE matmuls.

Per-core program:
  - qTp = Wq_g^T.T @ q^T          [hd on partitions, q free]
  - per kv chunk j (512 rows):
      kT chunk  = Wk_g^T.T @ kv^T chunk
      v chunk   = kv^T.T @ Wv_g^T  (+ ones column per head -> vA)
      per head: scoresT = kT_h.T @ qTp_h ; probsT = exp(scale*s + maskbias)
                av(+sums row) += vA_h.T @ probsT  (PSUM per chunk, fp32
                accumulated into SBUF attn_acc across chunks)
  - normalize per head-pair: recip of sums row, broadcast via tiny matmul,
    multiply -> attnT (bf16)
  - out = attnT.T @ Wo_g^T + bias ; DMA to cc_in
  - pairwise ReduceScatter(add) over q rows -> each core returns 256 q rows

kernel(**inputs) takes full inputs, shards/transposes/casts on host, runs SPMD
on cores 0-7, and reassembles the (4, 512, 1024) output.
"""

import numpy as np
import ml_dtypes

import concourse.bass as bass
import concourse.mybir as mybir
import concourse.tile as tile
from concourse import bacc
from concourse.bass_utils import run_bass_kernel_spmd

N_CORES = 8
P = 128
B, Q, KV, HID = 4, 512, 2048, 1024
HDS = 512          # head-dim slice per core (8 heads x 64)
NHEADS = 8         # heads per core
DH = 64
SCALE = 0.125      # 1/sqrt(64)
MASK_BIG = 1e30

F32 = mybir.dt.float32
BF16 = mybir.dt.bfloat16
F32R = mybir.dt.float32r

NKB = HID // P      # 8 contraction blocks
NMB = HDS // P      # 4 head-dim blocks (head pairs)
NCH = KV // 512     # 4 kv chunks


def _build(loop_k: int = 0, use_f32r: bool = True, analysis: bool = False,
           skip: tuple = ()):
    """Build the SPMD program. loop_k>0 wraps the compute in a For_i hardware
    loop (for timing); the collective + final DMAs stay outside the loop.
    analysis=True builds a 1-core, collective-free variant for TimelineSim.
    use_f32r=True -> bf16 operands (fast path); False -> f32r (debug).
    skip: timing-dissection only (results wrong): subsets of
    {"scores", "av", "norm", "out"}."""
    DT = BF16 if use_f32r else F32R
    do_proj = "proj" not in skip
    do_scores = do_proj and "scores" not in skip
    do_av = do_scores and "av" not in skip
    do_norm = do_av and "norm" not in skip
    do_out = do_proj and "out" not in skip

    nc = bacc.Bacc("TRN2", target_bir_lowering=False, debug=False,
                   num_devices=1 if analysis else N_CORES)

    qT_in = nc.dram_tensor("qT_in", [HID, Q], DT, kind="ExternalInput")
    kvT_in = nc.dram_tensor("kvT_in", [HID, KV], DT, kind="ExternalInput")
    wq_in = nc.dram_tensor("wqT", [HID, HDS], DT, kind="ExternalInput")
    wk_in = nc.dram_tensor("wkT", [HID, HDS], DT, kind="ExternalInput")
    wv_in = nc.dram_tensor("wvT", [HID, HDS], DT, kind="ExternalInput")
    wo_in = nc.dram_tensor("woT", [HDS, HID], DT, kind="ExternalInput")
    bias16_in = nc.dram_tensor("bias16", [P, KV // P], F32, kind="ExternalInput")
    biasbc_in = nc.dram_tensor("bias_bc", [P, HID], F32, kind="ExternalInput")
    onesc_in = nc.dram_tensor("onesc", [P, DH], DT, kind="ExternalInput")
    ones8_in = nc.dram_tensor("ones8", [P, NHEADS], DT, kind="ExternalInput")
    out_ext = nc.dram_tensor("out", [Q // 2, HID], F32, kind="ExternalOutput")

    cc_in = nc.dram_tensor("cc_in", [Q, HID], F32)
    cc_out = nc.dram_tensor("cc_out", [Q // 2, HID], F32)

    with tile.TileContext(nc) as tc:
        with (
            tc.tile_pool(name="persist", bufs=1) as pp,
            tc.tile_pool(name="probs", bufs=12) as prp,
            tc.tile_pool(name="small", bufs=2) as smallp,
            tc.tile_pool(name="outc", bufs=3) as ocp,
            tc.tile_pool(name="psum_proj", bufs=2, space="PSUM") as psp,
            tc.tile_pool(name="psum_s", bufs=4, space="PSUM") as pss,
            tc.tile_pool(name="psum_av", bufs=2, space="PSUM") as psav,
        ):
            def body():
                # ---- input DMAs (emission order ~= priority) ----
                wq = [pp.tile([P, HDS], DT, tag=f"wq{kb}", name=f"wq{kb}") for kb in range(NKB)]
                qT = [pp.tile([P, Q], DT, tag=f"qT{kb}", name=f"qT{kb}") for kb in range(NKB)]
                for kb in range(NKB):
                    nc.sync.dma_start(wq[kb][:], wq_in[kb * P:(kb + 1) * P, :])
                    nc.sync.dma_start(qT[kb][:], qT_in[kb * P:(kb + 1) * P, :])

                wk = [pp.tile([P, HDS], DT, tag=f"wk{kb}", name=f"wk{kb}") for kb in range(NKB)]
                wv = [pp.tile([P, HDS], DT, tag=f"wv{kb}", name=f"wv{kb}") for kb in range(NKB)]
                kvT = [pp.tile([P, KV], DT, tag=f"kvT{kb}", name=f"kvT{kb}") for kb in range(NKB)]
                for kb in range(NKB):
                    nc.sync.dma_start(wk[kb][:], wk_in[kb * P:(kb + 1) * P, :])
                for kb in range(NKB):
                    nc.sync.dma_start(kvT[kb][:, 0:512],
                                      kvT_in[kb * P:(kb + 1) * P, 0:512])
                for kb in range(NKB):
                    nc.sync.dma_start(wv[kb][:], wv_in[kb * P:(kb + 1) * P, :])

                bias16 = pp.tile([P, KV // P], F32, tag="bias16")
                nc.sync.dma_start(bias16[:], bias16_in[:, :])
                ones8 = pp.tile([P, NHEADS], DT, tag="ones8")
                nc.sync.dma_start(ones8[:], ones8_in[:, :])
                onesc = pp.tile([P, DH], DT, tag="onesc")
                nc.sync.dma_start(onesc[:], onesc_in[:, :])

                for j in range(1, NCH):
                    for kb in range(NKB):
                        nc.sync.dma_start(
                            kvT[kb][:, j * 512:(j + 1) * 512],
                            kvT_in[kb * P:(kb + 1) * P, j * 512:(j + 1) * 512])

                wo = [pp.tile([P, HID], DT, tag=f"wo{mb}", name=f"wo{mb}") for mb in range(NMB)]
                for mb in range(NMB):
                    nc.sync.dma_start(wo[mb][:], wo_in[mb * P:(mb + 1) * P, :])
                bias_bc = pp.tile([P, HID], F32, tag="bias_bc")
                nc.sync.dma_start(bias_bc[:], biasbc_in[:, :])

                # ---- q projection ----
                qTp = [pp.tile([P, Q], DT, tag=f"qTp{mb}", name=f"qTp{mb}") for mb in range(NMB)]
                for mb in range(NMB if do_proj else 0):
                    ps = psp.tile([P, 512], F32, tag="proj")
                    for kb in range(NKB):
                        nc.tensor.matmul(
                            ps[:], wq[kb][:, mb * P:(mb + 1) * P], qT[kb][:],
                            start=(kb == 0), stop=(kb == NKB - 1))
                    nc.vector.tensor_copy(out=qTp[mb][:], in_=ps[:])

                # ---- persistent attention state ----
                kT = [pp.tile([P, KV], DT, tag=f"kT{mb}", name=f"kT{mb}") for mb in range(NMB)]
                vA = [pp.tile([P, NHEADS * (DH + 1)], DT, tag=f"vA{kvb}", name=f"vA{kvb}")
                      for kvb in range(KV // P)]
                attn_acc = [pp.tile([DH + 1, Q], F32, tag=f"acc{h}", name=f"acc{h}")
                            for h in range(NHEADS)]
                attnT = [pp.tile([P, Q], DT, tag=f"attnT{mb}", name=f"attnT{mb}")
                         for mb in range(NMB)]

                # ---- kv chunks: k/v proj + attention ----
                for j in range(NCH if do_proj else 0):
                    # k-proj for this chunk
                    for mb in range(NMB):
                        ps = psp.tile([P, 512], F32, tag="proj")
                        for kb in range(NKB):
                            nc.tensor.matmul(
                                ps[:], wk[kb][:, mb * P:(mb + 1) * P],
                                kvT[kb][:, j * 512:(j + 1) * 512],
                                start=(kb == 0), stop=(kb == NKB - 1))
                        nc.vector.tensor_copy(
                            out=kT[mb][:, j * 512:(j + 1) * 512], in_=ps[:])
                    # v-proj for this chunk
                    for v4 in range(4):
                        kvb = 4 * j + v4
                        ps = psp.tile([P, 512], F32, tag="proj")
                        for kb in range(NKB):
                            nc.tensor.matmul(
                                ps[:], kvT[kb][:, kvb * P:(kvb + 1) * P],
                                wv[kb][:],
                                start=(kb == 0), stop=(kb == NKB - 1))
                        dst = vA[kvb][:].rearrange("p (h d) -> p h d", d=DH + 1)
                        nc.vector.tensor_copy(
                            out=dst[:, :, 0:DH],
                            in_=ps[:].rearrange("p (h d) -> p h d", d=DH))
                        nc.vector.tensor_copy(
                            out=dst[:, :, DH:DH + 1],
                            in_=ones8[:].rearrange("p (h o) -> p h o", o=1))

                    # attention for this chunk
                    for hp in range(NMB if do_scores else 0):
                        probs = {}
                        # scores interleaved by row-group pair (off 0 / 64)
                        # so the two 64-row matmuls can overlap on the PE
                        for k4 in range(4):
                            kvb = 4 * j + k4
                            for h2 in range(2):
                                off = h2 * DH
                                ss = pss.tile([P, Q], F32, tag="ss")
                                nc.tensor.matmul(
                                    ss[:],
                                    kT[hp][off:off + DH,
                                           kvb * P:(kvb + 1) * P],
                                    qTp[hp][off:off + DH, :],
                                    start=True, stop=True)
                                pr = prp.tile([P, Q], DT, tag="probs")
                                nc.scalar.activation(
                                    pr[:], ss[:],
                                    mybir.ActivationFunctionType.Exp,
                                    bias=bias16[:, kvb:kvb + 1], scale=SCALE)
                                probs[(h2, k4)] = pr
                        for h2 in range(2 if do_av else 0):
                            h = 2 * hp + h2
                            avp = psav.tile([DH + 1, Q], F32, tag="av")
                            for k4 in range(4):
                                kvb = 4 * j + k4
                                nc.tensor.matmul(
                                    avp[:],
                                    vA[kvb][:, h * (DH + 1):(h + 1) * (DH + 1)],
                                    probs[(h2, k4)][:],
                                    start=(k4 == 0), stop=(k4 == 3))
                            if j == 0:
                                nc.vector.tensor_copy(out=attn_acc[h][:],
                                                      in_=avp[:])
                            else:
                                nc.vector.tensor_tensor(
                                    attn_acc[h][:], avp[:], attn_acc[h][:],
                                    mybir.AluOpType.add)

                        # normalization per pair after last chunk
                        if j == NCH - 1 and do_norm:
                            recb = smallp.tile([DH + 1, Q], DT, tag="recb")
                            for h2 in range(2):
                                rf = smallp.tile([1, Q], F32,
                                                 tag=f"rf{h2}")
                                nc.vector.reciprocal(
                                    rf[:],
                                    attn_acc[2 * hp + h2][DH:DH + 1, :])
                                nc.vector.tensor_copy(
                                    out=recb[h2 * DH:h2 * DH + 1, :],
                                    in_=rf[:])
                            rps = psp.tile([P, 512], F32, tag="proj")
                            nc.tensor.matmul(rps[0:DH, :], onesc[0:1, :],
                                             recb[0:1, :],
                                             start=True, stop=True)
                            nc.tensor.matmul(rps[DH:P, :],
                                             onesc[DH:DH + 1, :],
                                             recb[DH:DH + 1, :],
                                             start=True, stop=True)
                            nc.vector.tensor_tensor(
                                attnT[hp][0:DH, :],
                                attn_acc[2 * hp][0:DH, :], rps[0:DH, :],
                                mybir.AluOpType.mult)
                            nc.vector.tensor_tensor(
                                attnT[hp][DH:P, :],
                                attn_acc[2 * hp + 1][0:DH, :], rps[DH:P, :],
                                mybir.AluOpType.mult)

                # ---- out projection + bias ----
                for qb in range(Q // P if do_out else 0):
                    for oh in range(2):
                        ps = psp.tile([P, 512], F32, tag="proj")
                        for mb in range(NMB):
                            nc.tensor.matmul(
                                ps[:], attnT[mb][:, qb * P:(qb + 1) * P],
                                wo[mb][:, oh * 512:(oh + 1) * 512],
                                start=(mb == 0), stop=(mb == NMB - 1))
                        oc = ocp.tile([P, 512], F32, tag="oc")
                        nc.vector.tensor_tensor(
                            oc[:], ps[:], bias_bc[:, oh * 512:(oh + 1) * 512],
                            mybir.AluOpType.add)
                        nc.sync.dma_start(
                            cc_in[qb * P:(qb + 1) * P,
                                  oh * 512:(oh + 1) * 512],
                            oc[:])

            if loop_k > 0:
                with tc.For_i(0, loop_k,
                              hint_engines=(mybir.EngineType.PE,
                                            mybir.EngineType.Activation,
                                            mybir.EngineType.DVE)):
                    body()
            else:
                body()

            # ---- pairwise reduce-scatter over q rows ----
            if analysis:
                nc.sync.dma_start(out_ext[:, :], cc_in[: Q // 2, :])
            else:
                nc.gpsimd.collective_compute(
                    "ReduceScatter",
                    mybir.AluOpType.add,
                    replica_groups=[[0, 1], [2, 3], [4, 5], [6, 7]],
                    ins=[cc_in.ap().opt()],
                    outs=[cc_out.ap().opt()],
                )
                nc.sync.dma_start(out_ext[:, :], cc_out[:, :])

    nc.compile()
    return nc


_CACHE = {}


def _get_nc(loop_k: int = 0, use_f32r: bool = True):
    key = (loop_k, use_f32r)
    if key not in _CACHE:
        _CACHE[key] = _build(loop_k, use_f32r)
    return _CACHE[key]


def make_in_maps(query, key_value, mask, Wq, Wk, Wv, Wo, bo, use_f32r=True):
    np_dt = ml_dtypes.bfloat16 if use_f32r else np.float32
    query = np.asarray(query, dtype=np.float32)
    key_value = np.asarray(key_value, dtype=np.float32)
    mask_f = np.asarray(mask).astype(np.float32)
    Wq = np.asarray(Wq, dtype=np.float32)
    Wk = np.asarray(Wk, dtype=np.float32)
    Wv = np.asarray(Wv, dtype=np.float32)
    Wo = np.asarray(Wo, dtype=np.float32)
    bo = np.asarray(bo, dtype=np.float32)

    onesc = np.ones((P, DH), dtype=np_dt)
    ones8 = np.ones((P, NHEADS), dtype=np_dt)

    in_maps = []
    for c in range(N_CORES):
        b, g = c // 2, c % 2
        sl = slice(g * HDS, (g + 1) * HDS)
        bias16 = ((mask_f[b] - 1.0) * MASK_BIG).reshape(KV // P, P).T
        bias_bc = np.broadcast_to(
            bo if g == 0 else np.zeros_like(bo), (P, HID))
        in_maps.append({
            "qT_in": np.ascontiguousarray(query[b].T).astype(np_dt),
            "kvT_in": np.ascontiguousarray(key_value[b].T).astype(np_dt),
            "wqT": np.ascontiguousarray(Wq[sl, :].T).astype(np_dt),
            "wkT": np.ascontiguousarray(Wk[sl, :].T).astype(np_dt),
            "wvT": np.ascontiguousarray(Wv[sl, :].T).astype(np_dt),
            "woT": np.ascontiguousarray(Wo[:, sl].T).astype(np_dt),
            "bias16": np.ascontiguousarray(bias16),
            "bias_bc": np.ascontiguousarray(bias_bc),
            "onesc": onesc,
            "ones8": ones8,
        })
    return in_maps


def kernel(query, key_value, mask, Wq, Wk, Wv, Wo, bo):
    nc = _get_nc(0, True)
    in_maps = make_in_maps(query, key_value, mask, Wq, Wk, Wv, Wo, bo)
    res = run_bass_kernel_spmd(nc, in_maps, list(range(N_CORES))).results
    out = np.empty((B, Q, HID), dtype=np.float32)
    for b_i in range(B):
        out[b_i, : Q // 2] = res[2 * b_i]["out"]
        out[b_i, Q // 2:] = res[2 * b_i + 1]["out"]
    return out
